# revision 15
# baseline (speedup 1.0000x reference)
"""SIREN MLP forward (nn_Neurcomp) on 8 TRN2 NeuronCores, pure data parallel.

Math per point (reference):
  h = sin(O*(x @ W0.T + b0))
  7 residual blocks: s1 = sin(O*((w1*h) @ Wa.T + ba)); s2 = sin(O*(s1 @ Wb.T + bb))
                     h = w2*(h + s2)        (w1=0.5 except blk0, w2=0.5 only last blk)
  y = h @ Wf.T + bf

Kernel strategy (per core, 32768 points, feature-major layout hT=[128, pts]):
  * All sine-layer weights pre-scaled by OMEGA/2pi (and w1 folds) so matmuls
    produce arguments in TURNS; w2 folds into Wf. Hidden weights pre-rounded to
    fp32r (11-bit mantissa) -> 1 cycle/row TensorE matmuls; products are exact.
  * Range reduction per sine layer:
      - DVE: v = bf16(psum + (b_u + 136))  -- bf16 RNE at ulp=1 rounds to the
        nearest integer => v = 136 + k, k = rne(zU + b_u)
      - PE:  psum += (-I)bf16 @ v          -- subtracts (k + 136) in PSUM
      - ACT: s = sin(2pi*psum + 2pi*(b_u+136))  [fp32r out], |arg| <= pi + eps
  * Residual adds on GpSimd (fp32r out), final 128->1 layer via fp32r matmul
    into psum row 0, bias via ACT copy.
"""
import sys

sys.path.insert(0, "/opt/trn_rl_repo")
import numpy as np
import ml_dtypes

import concourse.bass as bass
import concourse.tile as tile
from concourse import bacc, mybir
from concourse.bass_utils import run_bass_kernel_spmd

OMEGA = 30.0
N_PTS, D_IN, HID, NB = 262144, 3, 128, 7
NL = 2 * NB + 1  # 15 sine layers
N_CORES = 8
PPC = N_PTS // N_CORES  # 32768 points per core
ST = 1024  # supertile columns
N_ST = PPC // ST
CH = 512  # matmul moving-operand / psum-bank chunk
TWO_PI = 2.0 * np.pi
MAGIC = 136.0  # bf16 ulp=1 zone [128,256)

F32 = mybir.dt.float32
F32R = mybir.dt.float32r
BF16 = mybir.dt.bfloat16

# tuning knobs
ACT_V_LAYERS = frozenset({8})  # sine layers whose v-op runs on ACT instead of DVE
RESIDUAL_ENGINE = "gpsimd"  # or "vector"
STAGGER = 3  # rounds between chain launches

_CACHE = {}


def _round_fp32r(a):
    bits = np.ascontiguousarray(a, np.float32).view(np.uint32).astype(np.uint64)
    lsb = (bits >> np.uint64(12)) & np.uint64(1)
    r = (bits + np.uint64(0x7FF) + lsb) & np.uint64(0xFFFFF000)
    return r.astype(np.uint32).view(np.float32)


def _build():
    if "nc" in _CACHE:
        return _CACHE["nc"]
    nc = bacc.Bacc(
        "TRN2", target_bir_lowering=False, debug=False, num_devices=N_CORES
    )
    xT_ap = nc.dram_tensor("xT", [3 * D_IN, PPC], F32R, kind="ExternalInput").ap()
    w0_ap = nc.dram_tensor("W0U", [3 * D_IN, HID], F32R, kind="ExternalInput").ap()
    wh_ap = nc.dram_tensor("WH", [2 * NB, HID, HID], F32R, kind="ExternalInput").ap()
    wf_ap = nc.dram_tensor("WFU", [HID, 1], F32R, kind="ExternalInput").ap()
    ni_ap = nc.dram_tensor("NEGI", [HID, HID], BF16, kind="ExternalInput").ap()
    bu_ap = nc.dram_tensor("BU136", [HID, NL], F32, kind="ExternalInput").ap()
    b2_ap = nc.dram_tensor("B2PI", [HID, NL], F32, kind="ExternalInput").ap()
    bf_ap = nc.dram_tensor("BF", [1, 1], F32, kind="ExternalInput").ap()
    out_ap = nc.dram_tensor("out", [N_ST, ST], F32, kind="ExternalOutput").ap()

    Sin = mybir.ActivationFunctionType.Sin
    Copy = mybir.ActivationFunctionType.Identity

    with tile.TileContext(nc) as tc:
        with (
            tc.tile_pool(name="const", bufs=1) as cpool,
            tc.tile_pool(name="xin", bufs=6) as xpool,
            tc.tile_pool(name="hbuf", bufs=12) as hpool,
            tc.tile_pool(name="sbuf", bufs=7) as spool,
            tc.tile_pool(name="vbuf", bufs=8) as vpool,
            tc.tile_pool(name="obuf", bufs=4) as opool,
            tc.tile_pool(name="ps", bufs=4, space="PSUM") as pspool,
        ):
            w0 = cpool.tile([3 * D_IN, HID], F32R, tag="w0")
            nc.sync.dma_start(w0[:], w0_ap[:])
            wh = []
            for j in range(2 * NB):
                t = cpool.tile([HID, HID], F32R, tag=f"wh{j}")
                nc.sync.dma_start(t[:], wh_ap[j])
                wh.append(t)
            wfu = cpool.tile([HID, 1], F32R, tag="wfu")
            nc.sync.dma_start(wfu[:], wf_ap[:])
            negi = cpool.tile([HID, HID], BF16, tag="negi")
            nc.sync.dma_start(negi[:], ni_ap[:])
            bu = cpool.tile([HID, NL], F32, tag="bu")
            nc.sync.dma_start(bu[:], bu_ap[:])
            b2 = cpool.tile([HID, NL], F32, tag="b2")
            nc.sync.dma_start(b2[:], b2_ap[:])
            bft = cpool.tile([1, 1], F32, tag="bft")
            nc.sync.dma_start(bft[:], bf_ap[:])

            def emit_mains(lidx, lhsT, rhs):
                ps = pspool.tile([HID, ST], F32, tag="ps")
                for j in range(ST // CH):
                    nc.tensor.matmul(
                        ps[:, j * CH : (j + 1) * CH],
                        lhsT[:],
                        rhs[:, j * CH : (j + 1) * CH],
                        start=True,
                        stop=False,
                    )
                return ps

            def emit_v(lidx, ps):
                v = vpool.tile([HID, ST], BF16, tag="v")
                if lidx in ACT_V_LAYERS:
                    nc.scalar.activation(
                        v[:], ps[:], Copy, bias=bu[:, lidx : lidx + 1], scale=1.0
                    )
                else:
                    nc.vector.tensor_scalar_add(v[:], ps[:], bu[:, lidx : lidx + 1])
                return v

            def emit_kmm(ps, v):
                for j in range(ST // CH):
                    nc.tensor.matmul(
                        ps[:, j * CH : (j + 1) * CH],
                        negi[:],
                        v[:, j * CH : (j + 1) * CH],
                        start=False,
                        stop=True,
                    )

            def emit_sin(lidx, ps):
                s = spool.tile([HID, ST], F32R, tag=f"s{lidx % 2}")
                nc.scalar.activation(
                    s[:], ps[:], Sin, bias=b2[:, lidx : lidx + 1], scale=float(TWO_PI)
                )
                return s

            def residual(h, s2):
                hn = hpool.tile([HID, ST], F32R, tag="h")
                if RESIDUAL_ENGINE == "gpsimd":
                    nc.gpsimd.tensor_tensor(
                        hn[:],
                        h[:].bitcast(F32),
                        s2[:].bitcast(F32),
                        op=mybir.AluOpType.add,
                    )
                else:
                    nc.vector.tensor_tensor(
                        hn[:],
                        h[:].bitcast(F32),
                        s2[:].bitcast(F32),
                        op=mybir.AluOpType.add,
                    )
                return hn

            # Staggered wavefront: chain t starts its 16-stage pipeline at
            # round STAGGER*t, so ~16/STAGGER chains are always in flight at
            # DIFFERENT stages -- engines keep independent work, no group
            # boundary drains.
            N_STAGES = NL + 1  # 15 sine layers + final
            state = {}  # t -> dict(h=, s1=, s2=)

            def emit_final(t_i):
                # final: y = (h6 + s2_6) @ WFU + bf as two moving passes
                st = state[t_i]
                h, s2 = st["h"], st["s2"]
                psf = pspool.tile([HID, ST], F32, tag="ps")
                for j in range(ST // CH):
                    nc.tensor.matmul(
                        psf[0:1, j * CH : (j + 1) * CH],
                        wfu[:],
                        h[:, j * CH : (j + 1) * CH],
                        start=True,
                        stop=False,
                    )
                    nc.tensor.matmul(
                        psf[0:1, j * CH : (j + 1) * CH],
                        wfu[:],
                        s2[:, j * CH : (j + 1) * CH],
                        start=False,
                        stop=True,
                    )
                orow = opool.tile([1, ST], F32, tag="o")
                nc.scalar.activation(orow[:], psf[0:1, :], Copy, bias=bft[:], scale=1.0)
                nc.sync.dma_start(out_ap[t_i : t_i + 1, :], orow[:])
                del state[t_i]

            def layer_args(t_i, stage):
                st = state.setdefault(t_i, {})
                if stage == 0:
                    xt = xpool.tile([3 * D_IN, ST], F32R, tag="xt")
                    nc.sync.dma_start(xt[:], xT_ap[:, t_i * ST : (t_i + 1) * ST])
                    return w0, xt
                blk, phase = divmod(stage - 1, 2)
                if phase == 0:
                    return wh[2 * blk], st["h"]
                return wh[2 * blk + 1], st["s1"]

            def store_result(t_i, stage, s):
                st = state[t_i]
                if stage == 0:
                    st["h"] = s
                    return
                blk, phase = divmod(stage - 1, 2)
                if phase == 0:
                    st["s1"] = s
                elif blk < NB - 1:
                    st["h"] = residual(st["h"], s)
                else:
                    st["s2"] = s

            total_rounds = STAGGER * (N_ST - 1) + N_STAGES
            for r in range(total_rounds):
                active = []  # (t_i, stage) oldest chain (highest stage) first
                for t_i in range(N_ST):
                    stage = r - STAGGER * t_i
                    if 0 <= stage < N_STAGES:
                        active.append((t_i, stage))
                # phase-split emission: engines see independent work batched
                finals = [t for t, sg in active if sg == NL]
                layers = [(t, sg) for t, sg in active if sg < NL]
                work = []
                for t_i, sg in layers:
                    lhsT, rhs = layer_args(t_i, sg)
                    ps = emit_mains(sg, lhsT, rhs)
                    work.append((t_i, sg, ps))
                for t_i in finals:
                    emit_final(t_i)
                vs = [emit_v(sg, ps) for (t_i, sg, ps) in work]
                for (t_i, sg, ps), v in zip(work, vs):
                    emit_kmm(ps, v)
                for t_i, sg, ps in work:
                    s = emit_sin(sg, ps)
                    store_result(t_i, sg, s)

    nc.compile()
    _CACHE["nc"] = nc
    return nc


def _prep_in_maps(x, W0, b0, Wa, ba, Wb, bb, Wf, bf):
    f64 = np.float64
    scale = OMEGA / TWO_PI
    w1 = np.where(np.arange(NB) > 0, 0.5, 1.0)

    W0U_f = (scale * W0.astype(f64).T).astype(np.float32)  # [3, 128]
    W0_hi = _round_fp32r(W0U_f)
    W0_lo = _round_fp32r((W0U_f.astype(f64) - W0_hi).astype(np.float32))
    W0U = np.concatenate([W0_hi, W0_hi, W0_lo], axis=0)  # [9, 128]
    WH = np.empty((2 * NB, HID, HID), np.float32)
    for k in range(NB):
        WH[2 * k] = _round_fp32r((scale * w1[k] * Wa[k].astype(f64).T).astype(np.float32))
        WH[2 * k + 1] = _round_fp32r((scale * Wb[k].astype(f64).T).astype(np.float32))
    WFU = _round_fp32r((0.5 * Wf.astype(f64).T).astype(np.float32))  # [128, 1]
    NEGI = (-np.eye(HID)).astype(ml_dtypes.bfloat16)

    # per-layer biases in turns: [15, 128] -> [128, 15]
    b_u = np.empty((NL, HID), f64)
    b_u[0] = OMEGA * b0.astype(f64) / TWO_PI
    for k in range(NB):
        b_u[1 + 2 * k] = OMEGA * ba[k].astype(f64) / TWO_PI
        b_u[2 + 2 * k] = OMEGA * bb[k].astype(f64) / TWO_PI
    BU136 = (b_u.T + MAGIC).astype(np.float32)  # [128, 15]
    B2PI = (TWO_PI * (b_u.T + MAGIC)).astype(np.float32)
    BF = np.asarray(bf, np.float32).reshape(1, 1)

    common = {
        "W0U": np.ascontiguousarray(W0U),
        "WH": WH,
        "WFU": np.ascontiguousarray(WFU),
        "NEGI": NEGI,
        "BU136": np.ascontiguousarray(BU136),
        "B2PI": np.ascontiguousarray(B2PI),
        "BF": BF,
    }
    in_maps = []
    for c in range(N_CORES):
        xs = np.ascontiguousarray(x[c * PPC : (c + 1) * PPC].astype(np.float32).T)
        x_hi = _round_fp32r(xs)
        x_lo = _round_fp32r((xs.astype(f64) - x_hi).astype(np.float32))
        xT9 = np.concatenate([x_hi, x_lo, x_hi], axis=0)  # [9, PPC]
        in_maps.append({"xT": xT9, **common})
    return in_maps


def _run(in_maps, trace=False):
    nc = _build()
    res = run_bass_kernel_spmd(
        nc, in_maps, core_ids=list(range(N_CORES)), trace=trace
    )
    outs = [r["out"].reshape(-1) for r in res.results]
    full = np.concatenate(outs).reshape(N_PTS, 1).astype(np.float32)
    return full, res


def kernel(**inputs):
    in_maps = _prep_in_maps(**inputs)
    out, _ = _run(in_maps, trace=False)
    return out


def _install_ntff_hook():
    """The agent image's antenv lacks axon_hooks; synthesize it so
    run_bass_kernel_spmd(trace=True) can capture NTFF profiles."""
    import types
    import ctypes
    import contextlib

    try:
        from antenv.axon_hooks import get_axon_ntff_profile_hook  # noqa: F401

        return
    except ImportError:
        pass

    so_path = "/opt/axon/libaxon_pjrt.so"
    lib = ctypes.CDLL(so_path)
    if not hasattr(lib, "axon_start_nrt_profile"):
        return
    lib.axon_start_nrt_profile.argtypes = [
        ctypes.POINTER(ctypes.c_int64),
        ctypes.c_size_t,
    ]
    lib.axon_start_nrt_profile.restype = ctypes.c_int64
    lib.axon_stop_nrt_profile.argtypes = [ctypes.c_char_p]
    lib.axon_stop_nrt_profile.restype = ctypes.c_int64

    @contextlib.contextmanager
    def _hook(output_dir, device_ids):
        import jax

        jax.devices()
        if device_ids:
            ids = (ctypes.c_int64 * len(device_ids))(*device_ids)
            rc = lib.axon_start_nrt_profile(ids, len(device_ids))
        else:
            rc = lib.axon_start_nrt_profile(None, 0)
        if rc != 0:
            raise RuntimeError(f"axon_start_nrt_profile rc={rc}")
        try:
            yield
        finally:
            n = lib.axon_stop_nrt_profile(str(output_dir).encode())
            print(f"profile: {n} file(s) written to {output_dir}", file=sys.stderr)

    _hooks = {"h": _hook}
    mod = types.ModuleType("antenv.axon_hooks")
    mod.get_axon_ntff_profile_hook = lambda: _hooks["h"]
    mod.set_axon_ntff_profile_hook = lambda h: _hooks.__setitem__("h", h)
    sys.modules["antenv.axon_hooks"] = mod
    import antenv

    antenv.axon_hooks = mod


def kernel_profiled(**inputs):
    _install_ntff_hook()
    in_maps = _prep_in_maps(**inputs)
    out, res = _run(in_maps, trace=True)
    return out, res
